# revision 21
# baseline (speedup 1.0000x reference)
"""Fused decoder block (RMSNorm -> causal MHA -> proj+res -> RMSNorm -> FFN+res)
for Trainium2, 8 NeuronCores.

Sharding: tokens are row-sharded. Each core owns 512 query tokens (4 blocks of
128) of one batch, chosen so causal attention work is balanced: core j of the
batch group {0..3} owns blocks [15-j, 11-j, 7-j, 3-j]. Every core redundantly
computes K/V for its whole batch (zero collectives). A fixed padded attention
schedule (slot kv-lengths 16/12/8/4 blocks) keeps the program identical across
cores; causality/padding is carried entirely by per-core mask data.

Layout: activations are feature-major ("transposed", [C, tokens]) end to end,
so QKV/proj/FFN matmuls chain without transposes. Attention computes S^T
directly; softmax needs no max-subtraction (inputs are RMS-normed, |S| small);
denominators come free via a ones-column interleaved into V. Dense matmuls run
in float32r (TF32-like, full PE rate); attention operands (q/k/v/P) are bf16
with fp32 PSUM accumulation.
"""

import os
from contextlib import ExitStack

import ml_dtypes
import numpy as np

import concourse.tile as tile
from concourse import bacc, mybir
from concourse.bass_utils import run_bass_kernel_spmd

F32 = mybir.dt.float32
F32R = mybir.dt.float32r
BF16 = mybir.dt.bfloat16
AF = mybir.ActivationFunctionType

B, T, C = 2, 2048, 1024
H, HD = 16, 64
FF = 2 * C
EPS = float(np.finfo(np.float32).eps)
N_CORES = 8
NB = T // 128            # 16 token blocks per batch
LS = [16, 12, 8, 4]      # padded kv-length (blocks) per q-slot
NSLOT = 4
NOWN = NSLOT * 128       # 512 owned tokens per core
CCH = C // 128           # 8 feature chunks
FCH = FF // 128          # 16 ffn chunks
HP = H // 2              # 8 head pairs

LAST_EXEC_NS = None


def _active(kb):
    # active q-columns for kv-block kb: slots ordered by decreasing length
    return 128 * sum(1 for s in range(NSLOT) if kb < LS[s])


def _mask_coloff(kb):
    # the slot whose diagonal group contains kb (kb in [LS[s]-4, LS[s]))
    for s in range(NSLOT):
        if LS[s] - 4 <= kb < LS[s]:
            return 128 * s
    raise AssertionError(kb)


def _build_program():
    nc = bacc.Bacc("TRN2", target_bir_lowering=False, debug=False,
                   num_devices=N_CORES)

    d = {}
    d["tgtT"] = nc.dram_tensor("tgtT", [C, T], BF16, kind="ExternalInput")
    d["tgtTq"] = nc.dram_tensor("tgtTq", [C, NOWN], F32R, kind="ExternalInput")
    d["wqT"] = nc.dram_tensor("wqT", [C, C], BF16, kind="ExternalInput")
    d["wkT"] = nc.dram_tensor("wkT", [C, C], BF16, kind="ExternalInput")
    d["wvT"] = nc.dram_tensor("wvT", [C, C], BF16, kind="ExternalInput")
    d["wpT"] = nc.dram_tensor("wpT", [C, C], BF16, kind="ExternalInput")
    d["w1T"] = nc.dram_tensor("w1T", [C, FF], BF16, kind="ExternalInput")
    d["w2T"] = nc.dram_tensor("w2T", [FF, C], BF16, kind="ExternalInput")
    d["bp"] = nc.dram_tensor("bp", [128, CCH], F32, kind="ExternalInput")
    d["b1"] = nc.dram_tensor("b1", [128, FCH], F32, kind="ExternalInput")
    d["b2"] = nc.dram_tensor("b2", [128, CCH], F32, kind="ExternalInput")
    d["masks"] = nc.dram_tensor("masks", [NB * 128, 128], BF16,
                                kind="ExternalInput")
    d["onesv"] = nc.dram_tensor("onesv", [128, 1], F32R, kind="ExternalInput")
    d["tgtTqb"] = nc.dram_tensor("tgtTqb", [C, NOWN], BF16, kind="ExternalInput")
    d["outT"] = nc.dram_tensor("outT", [C, NOWN], F32, kind="ExternalOutput")
    kown_d = nc.dram_tensor("kown", [C, NOWN], BF16)
    vown_d = nc.dram_tensor("vown", [NOWN, H * 65], BF16)
    kall_d = nc.dram_tensor("kall", [4 * C, NOWN], BF16)
    vall_d = nc.dram_tensor("vall", [4 * NOWN, H * 65], BF16)

    with tile.TileContext(nc) as tc, ExitStack() as ctx:
        const = ctx.enter_context(tc.tile_pool(name="const", bufs=1))
        p_q = ctx.enter_context(tc.tile_pool(name="pq", bufs=1))
        p_kv = ctx.enter_context(tc.tile_pool(name="pkv", bufs=1))
        p_a = ctx.enter_context(tc.tile_pool(name="pa", bufs=1))
        p_x = ctx.enter_context(tc.tile_pool(name="px", bufs=2))
        work = ctx.enter_context(tc.tile_pool(name="work", bufs=2))
        work1 = ctx.enter_context(tc.tile_pool(name="work1", bufs=1))
        workp = ctx.enter_context(tc.tile_pool(name="workp", bufs=3))
        ps1 = ctx.enter_context(tc.tile_pool(name="ps1", bufs=1, space="PSUM"))
        ps2 = ctx.enter_context(tc.tile_pool(name="ps2", bufs=2, space="PSUM"))

        ones = const.tile([128, 1], F32R, tag="ones", name="ones")
        nc.sync.dma_start(ones[:], d["onesv"][:])
        bp_t = const.tile([128, CCH], F32, tag="bp", name="bp")
        b1_t = const.tile([128, FCH], F32, tag="b1", name="b1")
        b2_t = const.tile([128, CCH], F32, tag="b2", name="b2")
        nc.sync.dma_start(bp_t[:], d["bp"][:])
        nc.sync.dma_start(b1_t[:], d["b1"][:])
        nc.sync.dma_start(b2_t[:], d["b2"][:])
        mask_t = const.tile([128, NB * 128], BF16, tag="mask", name="mask")
        nc.sync.dma_start(
            mask_t[:].rearrange("p (k q) -> p k q", k=NB),
            d["masks"][:].rearrange("(k p) q -> p k q", p=128),
        )

        # rmsnorm in transposed layout, in place:
        # tiles[c] : [128, n] feature-major; tiles[c] *= rsqrt(mean+eps)
        def norm_t(tiles, n):
            ssq = ps1.tile([1, n], F32, tag="psA", name="ssq")
            for c in range(CCH):
                sq = work1.tile([128, 512], F32R, tag="sq", name="sq")
                nc.scalar.activation(sq[:, :n], tiles[c][:, :n], AF.Square)
                nc.tensor.matmul(ssq[:], ones[:], sq[:, :n],
                                 start=(c == 0), stop=(c == CCH - 1))
            m1 = work1.tile([1, 512], F32, tag="m1", name="m1")
            nc.scalar.activation(m1[:, :n], ssq[:], AF.Copy,
                                 bias=EPS, scale=1.0 / C)
            nc.vector.reciprocal(m1[:, :n], m1[:, :n])
            rs = work1.tile([1, 512], F32, tag="rs", name="rs")
            nc.scalar.activation(rs[:, :n], m1[:, :n], AF.Sqrt)
            rb = work1.tile([128, 512], F32, tag="rb", name="rb")
            nc.gpsimd.partition_broadcast(rb[:, :n], rs[:, :n])
            for c in range(CCH):
                nc.vector.tensor_mul(tiles[c][:, :n], tiles[c][:, :n],
                                     rb[:, :n])

        # persistent attention-state tiles
        qT = [p_q.tile([128, NOWN], BF16, tag=f"q{hp}", name=f"q{hp}")
              for hp in range(HP)]
        kT = [[p_kv.tile([128, 512], BF16, tag=f"k{hp}_{st}",
                         name=f"k{hp}_{st}") for st in range(4)]
              for hp in range(HP)]
        vp = [p_kv.tile([128, H * 65], BF16, tag=f"v{t}", name=f"v{t}")
              for t in range(NB)]
        aT = [p_a.tile([128, NOWN], BF16, tag=f"aT{hp}", name=f"aT{hp}")
              for hp in range(HP)]
        raw = [[p_a.tile([65, NOWN], F32, tag=f"rw{hp}_{h}",
                         name=f"rw{hp}_{h}") for h in range(2)]
               for hp in range(HP)]

        xT = [p_x.tile([128, 512], BF16, tag=f"x{c}", name=f"xq{c}")
              for c in range(CCH)]

        with tc.tile_pool(name="pw", bufs=2) as p_w:
            # ---- Q projection for own tokens (xq reuses the x tags) ----
            wq_t = []
            for c in range(CCH):
                w = p_w.tile([128, C], BF16, tag=f"pw{c}", name=f"wq{c}")
                nc.sync.dma_start(w[:], d["wqT"][c * 128:(c + 1) * 128, :])
                wq_t.append(w)
            for c in range(CCH):
                nc.sync.dma_start(xT[c][:], d["tgtTqb"][c * 128:(c + 1) * 128, :])
            norm_t(xT, NOWN)
            for hp in range(HP):
                acc = ps2.tile([128, 512], F32, tag="big", name="big")
                for c in range(CCH):
                    nc.tensor.matmul(acc[:], wq_t[c][:, hp * 128:(hp + 1) * 128],
                                     xT[c][:], start=(c == 0),
                                     stop=(c == CCH - 1))
                nc.vector.tensor_copy(qT[hp][:], acc[:])

            # ---- K/V for OWN blocks only, then AllGather in batch group ----
            wv_t, wk_t = [], []
            for c in range(CCH):
                w = p_w.tile([128, C], BF16, tag=f"pw{c}", name=f"wv{c}")
                nc.sync.dma_start(w[:], d["wvT"][c * 128:(c + 1) * 128, :])
                wv_t.append(w)
            for c in range(CCH):
                w = p_w.tile([128, C], BF16, tag=f"pw{c}", name=f"wk{c}")
                nc.sync.dma_start(w[:], d["wkT"][c * 128:(c + 1) * 128, :])
                wk_t.append(w)

            for hp in range(HP):
                acc = ps2.tile([128, 512], F32, tag="big", name="big")
                for c in range(CCH):
                    nc.tensor.matmul(acc[:], wk_t[c][:, hp * 128:(hp + 1) * 128],
                                     xT[c][:], start=(c == 0),
                                     stop=(c == CCH - 1))
                ko = work1.tile([128, NOWN], BF16, tag="ko", name=f"ko{hp}")
                nc.scalar.copy(ko[:], acc[:])
                nc.sync.dma_start(kown_d[hp * 128:(hp + 1) * 128, :], ko[:])
            for tb in range(NSLOT):
                vo = work1.tile([128, H * 65], BF16, tag="vo", name=f"vo{tb}")
                nc.gpsimd.memset(
                    vo[:].rearrange("p (h x) -> p h x", x=65)[:, :, 64:65], 1.0)
                for vh in range(2):
                    acc = ps2.tile([128, 512], F32, tag="big", name="big")
                    for c in range(CCH):
                        nc.tensor.matmul(
                            acc[:], xT[c][:, tb * 128:(tb + 1) * 128],
                            wv_t[c][:, vh * 512:(vh + 1) * 512],
                            start=(c == 0), stop=(c == CCH - 1))
                    nc.vector.tensor_copy(
                        vo[:].rearrange("p (h x) -> p h x", x=65)
                        [:, 8 * vh:8 * vh + 8, 0:64],
                        acc[:].rearrange("p (h e) -> p h e", e=64))
                nc.sync.dma_start(vown_d[tb * 128:(tb + 1) * 128, :], vo[:])

            groups = [[0, 1, 2, 3], [4, 5, 6, 7]]
            nc.gpsimd.collective_compute(
                "AllGather", mybir.AluOpType.bypass,
                ins=[kown_d[:]], outs=[kall_d[:]], replica_groups=groups)
            nc.gpsimd.collective_compute(
                "AllGather", mybir.AluOpType.bypass,
                ins=[vown_d[:]], outs=[vall_d[:]], replica_groups=groups)

            # load gathered K/V: owner j's image rows [j*C ...]; kb ->
            # owner (15-kb)%4, slot (15-kb)//4
            for hp in range(HP):
                for j in range(4):
                    nc.sync.dma_start(
                        kT[hp][j][:],
                        kall_d[j * C + hp * 128: j * C + (hp + 1) * 128, :])
            for kb in range(NB):
                j, sl = (15 - kb) % 4, (15 - kb) // 4
                nc.sync.dma_start(
                    vp[kb][:],
                    vall_d[j * NOWN + sl * 128: j * NOWN + (sl + 1) * 128, :])

            # ---- attention: single pass over all 16 kv blocks ----
            for hp in range(HP):
                at_ps = [ps1.tile([65, NOWN], F32, tag=("psA", "at1")[h],
                                  name=f"at{hp}_{h}") for h in range(2)]
                for kb in range(NB):
                    act = _active(kb)
                    moff = _mask_coloff(kb)
                    for h in range(2):
                        st = ps2.tile([128, 512], F32, tag=f"st{h}",
                                      name=f"st{hp}_{kb}_{h}")
                        nc.tensor.matmul(
                            st[:, :act],
                            kT[hp][(15 - kb) % 4][64 * h:64 * h + 64,
                                   ((15 - kb) // 4) * 128:
                                   ((15 - kb) // 4) * 128 + 128],
                            qT[hp][64 * h:64 * h + 64, :act],
                            start=True, stop=True)
                        pt = workp.tile([128, 512], BF16, tag=f"pt{h}",
                                        name=f"pt{hp}_{kb}_{h}")
                        nc.scalar.activation(pt[:, :act], st[:, :act],
                                             AF.Exp, scale=HD ** -0.5)
                        nc.vector.tensor_mul(
                            pt[:, moff:moff + 128],
                            pt[:, moff:moff + 128],
                            mask_t[:, kb * 128:(kb + 1) * 128])
                        nc.tensor.matmul(
                            at_ps[h][:, :act],
                            vp[kb][:, 65 * (2 * hp + h):65 * (2 * hp + h) + 65],
                            pt[:, :act],
                            start=(kb == 0), stop=(kb == NB - 1),
                            skip_group_check=True)
                for h in range(2):
                    nc.vector.tensor_copy(raw[hp][h][:], at_ps[h][:])

            # deferred, batched normalize (keeps recips off the attn path)
            for hp in range(HP):
                for h in range(2):
                    den = work1.tile([1, 512], F32, tag="m1",
                                     name=f"den{hp}_{h}")
                    nc.vector.tensor_copy(den[:], raw[hp][h][64:65, :])
                    nc.vector.reciprocal(den[:], den[:])
                    rb = work1.tile([64, 512], F32, tag="rb",
                                    name=f"arb{hp}_{h}")
                    nc.gpsimd.partition_broadcast(rb[:], den[:])
                    nc.vector.tensor_mul(aT[hp][64 * h:64 * h + 64, :],
                                         raw[hp][h][0:64, :], rb[:])

            # ---- proj + bias + residual (wp/tgq reuse pw and v tags) ----
            wp_t, tgq = [], []
            for hp in range(HP):
                w = p_w.tile([128, C], BF16, tag=f"pw{hp}", name=f"wp{hp}")
                nc.sync.dma_start(w[:], d["wpT"][hp * 128:(hp + 1) * 128, :])
                wp_t.append(w)
            for c in range(CCH):
                x = p_kv.tile([128, NOWN], F32R, tag=f"v{8 + c}",
                              name=f"tg{c}")
                nc.sync.dma_start(x[:], d["tgtTq"][c * 128:(c + 1) * 128, :])
                tgq.append(x)
            res = [p_x.tile([128, 512], F32, tag=f"x{c}", name=f"res{c}")
                   for c in range(CCH)]
            for c in range(CCH):
                acc = ps2.tile([128, 512], F32, tag="big", name="big")
                for hp in range(HP):
                    nc.tensor.matmul(acc[:],
                                     wp_t[hp][:, c * 128:(c + 1) * 128],
                                     aT[hp][:], start=(hp == 0),
                                     stop=(hp == HP - 1))
                tmp = work.tile([128, 512], F32, tag="tmp", name=f"ptmp{c}")
                nc.scalar.activation(tmp[:], acc[:], AF.Identity,
                                     bias=bp_t[:, c:c + 1])
                nc.vector.tensor_add(res[c][:], tmp[:], tgq[c][:])

        # ---- norm2 -> yT (reuses aT tags), FFN ----
        yT = [p_x.tile([128, NOWN], BF16, tag=f"x{c}", name=f"y{c}")
              for c in range(CCH)]
        for c in range(CCH):
            nc.vector.tensor_copy(yT[c][:], res[c][:])
        norm_t(yT, NOWN)

        hT = ([p_kv.tile([128, NOWN], BF16, tag=f"k{f}_0", name=f"h{f}")
               for f in range(CCH)]
              + [p_kv.tile([128, NOWN], BF16, tag=f"k{f}_1", name=f"h{8 + f}")
                 for f in range(CCH)])
        with tc.tile_pool(name="pw1", bufs=1) as p_w1:
            w1_t = []
            for c in range(CCH):
                w = p_w1.tile([128, FF], BF16, tag=f"w1{c}", name=f"w1{c}")
                nc.sync.dma_start(w[:], d["w1T"][c * 128:(c + 1) * 128, :])
                w1_t.append(w)
            for f in range(FCH):
                acc = ps2.tile([128, 512], F32, tag="big", name="big")
                for c in range(CCH):
                    nc.tensor.matmul(acc[:], w1_t[c][:, f * 128:(f + 1) * 128],
                                     yT[c][:], start=(c == 0),
                                     stop=(c == CCH - 1))
                nc.scalar.activation(hT[f][:], acc[:], AF.Gelu,
                                     bias=b1_t[:, f:f + 1])

        with tc.tile_pool(name="pw2", bufs=1) as p_w2:
            w2_t = []
            for f in range(FCH):
                w = p_w2.tile([128, C], BF16, tag=f"w2{f}", name=f"w2{f}")
                nc.sync.dma_start(w[:], d["w2T"][f * 128:(f + 1) * 128, :])
                w2_t.append(w)
            for c in range(CCH):
                acc = ps2.tile([128, 512], F32, tag="big", name="big")
                for f in range(FCH):
                    nc.tensor.matmul(acc[:], w2_t[f][:, c * 128:(c + 1) * 128],
                                     hT[f][:], start=(f == 0),
                                     stop=(f == FCH - 1))
                tmp = work.tile([128, 512], F32, tag="tmp", name=f"otmp{c}")
                nc.scalar.activation(tmp[:], acc[:], AF.Identity,
                                     bias=b2_t[:, c:c + 1])
                o = work.tile([128, 512], F32, tag="o", name=f"o{c}")
                nc.vector.tensor_add(o[:], tmp[:], res[c][:])
                nc.sync.dma_start(d["outT"][c * 128:(c + 1) * 128, :], o[:])

    nc.compile()
    return nc


_NC = None


def _get_program():
    global _NC
    if _NC is None:
        _NC = _build_program()
    return _NC


def _core_blocks(j):
    return [15 - j, 11 - j, 7 - j, 3 - j]


def kernel(target, wq, wk, wv, w_proj, b_proj, w1, b1, w2, b2, g1, g2):
    global LAST_EXEC_NS
    nc = _get_program()

    f32 = np.float32
    target = np.asarray(target, f32)
    g1 = np.asarray(g1, f32)
    g2 = np.asarray(g2, f32)
    wq_all = np.asarray(wq, f32).reshape(H * HD, C)
    wk_all = np.asarray(wk, f32).reshape(H * HD, C)
    wv_all = np.asarray(wv, f32).reshape(H * HD, C)
    wqT = np.ascontiguousarray((wq_all * g1[None, :]).T).astype(ml_dtypes.bfloat16)
    wkT = np.ascontiguousarray((wk_all * g1[None, :]).T).astype(ml_dtypes.bfloat16)
    wvT = np.ascontiguousarray((wv_all * g1[None, :]).T).astype(ml_dtypes.bfloat16)
    wpT = np.ascontiguousarray(np.asarray(w_proj, f32).T).astype(ml_dtypes.bfloat16)
    w1T = np.ascontiguousarray((np.asarray(w1, f32) * g2[None, :]).T).astype(ml_dtypes.bfloat16)
    w2T = np.ascontiguousarray(np.asarray(w2, f32).T).astype(ml_dtypes.bfloat16)
    bp_h = np.ascontiguousarray(np.asarray(b_proj, f32).reshape(CCH, 128).T)
    b1_h = np.ascontiguousarray(np.asarray(b1, f32).reshape(FCH, 128).T)
    b2_h = np.ascontiguousarray(np.asarray(b2, f32).reshape(CCH, 128).T)

    ki = np.arange(128)[:, None]
    qj = np.arange(128)[None, :]
    in_maps = []
    for core in range(N_CORES):
        b, j = divmod(core, 4)
        ms = _core_blocks(j)
        tgtT = np.ascontiguousarray(target[b].T).astype(ml_dtypes.bfloat16)
        tq = np.concatenate([target[b, m * 128:(m + 1) * 128, :] for m in ms], 0)
        tgtTq = np.ascontiguousarray(tq.T)
        masks = np.zeros((NB, 128, 128), f32)
        for s, m in enumerate(ms):
            for kb in range(LS[s] - 4, LS[s]):
                masks[kb] = ((kb * 128 + ki) <= (m * 128 + qj)).astype(f32)
        in_maps.append({
            "tgtT": tgtT, "tgtTq": tgtTq,
            "tgtTqb": tgtTq.astype(ml_dtypes.bfloat16),
            "wqT": wqT, "wkT": wkT, "wvT": wvT, "wpT": wpT,
            "w1T": w1T, "w2T": w2T,
            "bp": bp_h, "b1": b1_h, "b2": b2_h,
            "onesv": np.ones((128, 1), f32),
            "masks": masks.reshape(NB * 128, 128).astype(ml_dtypes.bfloat16),
        })

    res = run_bass_kernel_spmd(nc, in_maps, list(range(N_CORES)),
                               trace=bool(os.environ.get("BASS_TRACE")))
    LAST_EXEC_NS = res.exec_time_ns

    out = np.empty((B, T, C), f32)
    for core in range(N_CORES):
        b, j = divmod(core, 4)
        o = res.results[core]["outT"].T  # [512, 1024]
        for s, m in enumerate(_core_blocks(j)):
            out[b, m * 128:(m + 1) * 128, :] = o[s * 128:(s + 1) * 128, :]
    return out
